# revision 33
# baseline (speedup 1.0000x reference)
"""Multi-head attention (B=2, S=2048, D=1024, H=16, d_k=64) on 8 TRN2 NeuronCores.

Sharding: head-parallel. Core c owns heads (2c, 2c+1) for both batch rows:
 - replicated inputs: qT/kT/vT = x.reshape(B*S, D).T  in bf16, [1024, 4096]
   (D on partitions so the TensorEngine contracts over D with no transposes)
 - per-core weights: Wq columns / Wo rows for its two heads
 - per-core output: partial = attn_out(own heads) @ Wo[own rows]  [4096, 1024] bf16
   The host sums the 8 partials (f32) and adds bo.  No cross-core comm.

Per-core dataflow (bf16 matmuls, f32 PSUM):
 1. qhT/khT [128, 2048] per batch = Wq_c.T @ xT (+bq), computed per
    512-col s-stripe with d-inner accumulation (8 back-to-back matmuls
    hide LDWEIGHTS).  Inputs arrive as [128, 8, 512] stripes — one
    3D-AP DMA per stripe — ordered q_s0, k_s0..s3, v, q_s1..s3 so the
    first scores fire ~14us in instead of ~40us.
    vh [2048, 130] natural = vT.T @ Wqv_c; Wqv has zero-cols / bqv has
    1.0-cols so each head gets a ones column -> attn@V also produces the
    softmax denominators.
 2. scoresT[t,s] = khT.T @ qhT, both heads packed into disjoint PE
    row-groups (K=64).  exp(x/8) on ScalarE from PSUM, bf16 out.
 3. attn@V accumulated over t; row 64 = denominator.  Normalize via
    f32r denominator broadcast (K=1 matmul) + fast reciprocal + mul.
 4. partial[s, :] = outT.T @ Wo_c -> bf16 -> DRAM.

Scheduling: ScalarE (exp, ~142us) and the TensorEngine are both
near-saturated, so emission order keeps ACT fed: each s-chunk's
scores+exp loop is emitted first; its attn@V/normalize/out-proj are
deferred one s-chunk and re-emitted between later score iterations via a
two-lane work queue (lane A: DMA-gated projection work with
earliest-iteration thresholds; lane B: deferred attention, gated only
for chunk 0 where it races the vh0 projections).
"""

import numpy as np
import ml_dtypes

B, S, D, H, DK = 2, 2048, 1024, 16, 64
NCORES = 8
HPC = H // NCORES          # heads per core = 2
BS = B * S                 # 4096
HD = HPC * DK              # 128 = per-core head dims

_cache = {}


def _build():
    import concourse.bass as bass
    import concourse.tile as tile
    from concourse import bacc, mybir

    f32 = mybir.dt.float32
    f32r = mybir.dt.float32r
    bf16 = mybir.dt.bfloat16
    Exp = mybir.ActivationFunctionType.Exp

    nc = bacc.Bacc("TRN2", target_bir_lowering=False, debug=False,
                   num_devices=NCORES)

    qT = nc.declare_dram_parameter("qT", [D, BS], bf16, isOutput=False)
    kT = nc.declare_dram_parameter("kT", [D, BS], bf16, isOutput=False)
    vT = nc.declare_dram_parameter("vT", [D, BS], bf16, isOutput=False)
    wq = nc.declare_dram_parameter("wq", [D, HD], bf16, isOutput=False)
    wqv = nc.declare_dram_parameter("wqv", [D, 130], bf16, isOutput=False)
    bq = nc.declare_dram_parameter("bq", [1, HD], f32, isOutput=False)
    bqv = nc.declare_dram_parameter("bqv", [1, 130], f32, isOutput=False)
    wo = nc.declare_dram_parameter("wo", [HD, D], bf16, isOutput=False)
    out = nc.declare_dram_parameter("out", [BS, D], bf16, isOutput=True)

    ND = D // 128            # 8 d-chunks
    NT = S // 128            # 16 t-chunks per batch
    NSC = S // 512           # 4 s-chunks per batch
    NST = 4                  # 512-col input stripes per batch

    with tile.TileContext(nc) as tc:
        with (
            tc.tile_pool(name="const", bufs=1) as pc,
            tc.tile_pool(name="xin", bufs=12) as pin,
            tc.tile_pool(name="proj", bufs=2) as pproj,
            tc.tile_pool(name="vh", bufs=2) as pvh,
            tc.tile_pool(name="exp", bufs=20) as pexp,
            tc.tile_pool(name="outT", bufs=2) as poutT,
            tc.tile_pool(name="small", bufs=2) as psmall,
            tc.tile_pool(name="ob", bufs=4) as pob,
            tc.tile_pool(name="ps", bufs=2, space="PSUM") as pps,
        ):
            # ---- constants + first input stripes, ALL on the sync queue
            # in priority order (two queues just split HBM bandwidth and
            # delay the critical path): q_s0, bq, wq unblock the first
            # projection; k_s0 the first scores; the rest follows.
            def dma_stripe(src, b, n, j, eng=None):
                """One 512-col stripe of an input, all 8 d-chunks, via a
                single 3D-AP DMA: tile[p, c, s] = src[c*128+p, b*S+j*512+s].
                NB these strided DMAs BLOCK their queue ~3us each (after a
                ~3-deep descriptor ring), so queue order IS the schedule."""
                t = pin.tile([128, ND, 512], bf16, tag="xs",
                             name=f"x{n}{b}{j}")
                ap = src[:, b * S + j * 512:b * S + (j + 1) * 512].rearrange(
                    "(c p) s -> p c s", p=128)
                (eng or nc.sync).dma_start(t[:, :, :], ap)
                return t

            qs0 = [None] * NST
            ks0 = [None] * NST
            qs0[0] = dma_stripe(qT, 0, "q", 0)
            bq_row = pc.tile([1, HD], f32)
            nc.sync.dma_start(bq_row[:], bq[:, :])
            wq_sb = pc.tile([128, ND, HD], bf16)
            nc.sync.dma_start(
                wq_sb[:, :, :],
                wq[:, :].rearrange("(c p) h -> p c h", p=128))
            ks0[0] = dma_stripe(kT, 0, "k", 0)
            bqv_row = pc.tile([1, 130], f32)
            nc.sync.dma_start(bqv_row[:], bqv[:, :])
            wqv_sb = pc.tile([128, ND, 130], bf16)
            nc.sync.dma_start(
                wqv_sb[:, :, :],
                wqv[:, :].rearrange("(c p) h -> p c h", p=128))
            wo_sb = pc.tile([HD, D], bf16)
            nc.sync.dma_start(wo_sb[:], wo[:, :])
            ks0[1] = dma_stripe(kT, 0, "k", 1)
            qs0[1] = dma_stripe(qT, 0, "q", 1)
            ks0[2] = dma_stripe(kT, 0, "k", 2)
            ks0[3] = dma_stripe(kT, 0, "k", 3)
            vt0 = [dma_stripe(vT, 0, "v", j) for j in range(NST)]
            qs0[2] = dma_stripe(qT, 0, "q", 2)
            qs0[3] = dma_stripe(qT, 0, "q", 3)

            ones_f = pc.tile([1, 128], f32)
            nc.vector.memset(ones_f[:], 1.0)
            ones_r = pc.tile([1, 128], f32r)
            nc.vector.tensor_copy(ones_r[:], ones_f[:])
            bq_row_r = pc.tile([1, HD], f32r)
            nc.vector.tensor_copy(bq_row_r[:], bq_row[:])
            bqv_row_r = pc.tile([1, 130], f32r)
            nc.vector.tensor_copy(bqv_row_r[:], bqv_row[:])

            # bq as per-partition column (qhT/khT bias) and broadcast
            # across partitions (vh bias, with the 1.0 ones-columns)
            ps_t = pps.tile([128, 128], f32, tag="p1")
            nc.tensor.matmul(ps_t, bq_row_r[:], ones_r[:],
                             start=True, stop=True)
            bq_col = pc.tile([128, 1], f32)
            nc.vector.tensor_copy(bq_col[:], ps_t[:, 0:1])
            ps_t2 = pps.tile([128, 130], f32, tag="p1")
            nc.tensor.matmul(ps_t2, ones_r[:], bqv_row_r[:],
                             start=True, stop=True)
            bqv_bc = pc.tile([128, 130], f32)
            nc.vector.tensor_copy(bqv_bc[:], ps_t2[:])

            def proj_stripe(xs, sb, j):
                """qh/kh for one 512-col stripe: 8 d-inner accumulating
                matmuls (LDWEIGHTS fully hidden) + bias."""
                ps = pps.tile([128, 512], f32, tag="p1", name="pjs")
                for d in range(ND):
                    nc.tensor.matmul(ps, wq_sb[:, d, :], xs[:, d, :],
                                     start=(d == 0), stop=(d == ND - 1))
                nc.vector.tensor_scalar_add(sb[:, j * 512:(j + 1) * 512],
                                            ps, bq_col[:])

            def proj_stripe_items(xs_of, sb_of, j, gate):
                """proj_stripe split into two 4-MM lane-A items so a
                single pump never displaces scores by more than ~1us
                (ACT's PSUM backlog is only 2 tiles deep)."""
                cell = {}

                def p_a():
                    ps = pps.tile([128, 512], f32, tag="p1", name="pjs")
                    xs = xs_of()
                    for d in range(4):
                        nc.tensor.matmul(ps, wq_sb[:, d, :], xs[:, d, :],
                                         start=(d == 0), stop=False)
                    cell["ps"] = ps

                def p_b():
                    ps = cell["ps"]
                    xs = xs_of()
                    for d in range(4, ND):
                        nc.tensor.matmul(ps, wq_sb[:, d, :], xs[:, d, :],
                                         start=False, stop=(d == ND - 1))
                    nc.vector.tensor_scalar_add(
                        sb_of()[:, j * 512:(j + 1) * 512], ps, bq_col[:])
                return [(gate, p_a), (gate, p_b)]

            def vh_items(b, hold, gates):
                items = []
                for t in range(NT):
                    def tt(t=t):
                        ps = pps.tile([128, 130], f32,
                                      tag="p1", name=f"pvh{t}")
                        vt = hold["vt" + str(b)][t // 4]
                        c0 = (t % 4) * 128
                        for d in range(ND):
                            nc.tensor.matmul(ps, vt[:, d, c0:c0 + 128],
                                             wqv_sb[:, d, :],
                                             start=(d == 0), stop=(d == ND - 1))
                        nc.vector.tensor_add(hold["vh" + str(b)][:, t, :],
                                             ps[:], bqv_bc[:])
                    items.append((gates[t], tt))
                return items

            # ---- two-lane deferred work queue ----
            laneA = []   # (min_iter, thunk): DMA-gated projection work
            laneB = []   # (min_iter, thunk): deferred attention work
            it = [0]

            def pump():
                popped = 0
                if laneA and laneA[0][0] <= it[0]:
                    laneA.pop(0)[1]()
                    popped = 1
                for _ in range(2 - popped):
                    if laneB and laneB[0][0] <= it[0]:
                        laneB.pop(0)[1]()
                it[0] += 1

            def defer_attnv(b, sc, exs, vh_of, oT, av_gates=None):
                ssl = slice(sc * 512, (sc + 1) * 512)
                cell = {}
                # item i = one head (i%2), FOUR consecutive t-tiles: all
                # MMs accumulate into the SAME PSUM bank back-to-back,
                # which is the only pattern where LDWEIGHTS stays hidden
                for i in range(NT // 2):
                    def av(i=i):
                        h = i % 2
                        if i == 0:
                            cell["att"] = [
                                pps.tile([65, 512], f32, tag="att",
                                         name=f"att{b}{sc}{hh}")
                                for hh in range(HPC)]
                        vh = vh_of()
                        for t in range(4 * (i // 2), 4 * (i // 2) + 4):
                            nc.tensor.matmul(cell["att"][h],
                                             vh[:, t, h * 65:h * 65 + 65],
                                             exs[t][:, h, :],
                                             start=(t == 0), stop=(t == NT - 1))
                    laneB.append((av_gates[i] if av_gates else 0, av))

                # norm and out-proj split into ~1us items so a single pump
                # never displaces scores past the 2-tile ACT backlog.
                # den broadcast runs on GpSimd (its queue only carries
                # ~0.65us out-DMA issues now), freeing 16 fp32 PE matmuls.
                # NB partition_broadcast/reciprocal_approx_fast misread at
                # base_partition != 0: copy the den row to a base-0 tile,
                # broadcast raw, recip after.
                def norm_h(h):
                    asb = psmall.tile([65, 512], f32, tag="asb",
                                      bufs=4, name=f"asb{h}")
                    nc.vector.tensor_copy(asb[:], cell["att"][h][:])
                    den = psmall.tile([1, 512], f32, tag="rec")
                    nc.vector.tensor_copy(den[:], asb[64:65, :])
                    bcd = psmall.tile([64, 512], f32, tag="bcd")
                    nc.gpsimd.partition_broadcast(bcd[:], den[0:1, :])
                    bcs = psmall.tile([64, 512], f32, tag="bcs")
                    nc.vector.reciprocal_approx_fast(bcs[:], bcd[:])
                    nc.vector.tensor_mul(oT[h * 64:(h + 1) * 64, ssl],
                                         asb[0:64, :], bcs[:])
                laneB.append((0, lambda: norm_h(0)))
                laneB.append((0, lambda: norm_h(1)))

                for g in range(4):
                    def op(g=g):
                        for u in range(2):
                            s1 = sc * 4 + g
                            n = u
                            s0 = s1 * 128
                            rs = slice(b * S + s0, b * S + s0 + 128)
                            nsl = slice(n * 512, (n + 1) * 512)
                            ps = pps.tile([128, 512], f32,
                                          tag="p1", name="oppsum")
                            nc.tensor.matmul(ps, oT[:, s0:s0 + 128],
                                             wo_sb[:, nsl],
                                             start=True, stop=True)
                            ob = pob.tile([128, 512], bf16, tag="ob")
                            nc.vector.tensor_copy(ob[:], ps)
                            nc.gpsimd.dma_start(out[rs, nsl], ob[:])
                    laneB.append((0, op))

            def inline_tail(b, sc, att, oT):
                ssl = slice(sc * 512, (sc + 1) * 512)
                asbs = []
                for h in range(HPC):
                    asb = psmall.tile([65, 512], f32, tag="asb",
                                      bufs=4, name=f"asbL{h}")
                    nc.vector.tensor_copy(asb[:], att[h][:])
                    asbs.append(asb)
                for h in range(HPC):
                    den_r = psmall.tile([1, 512], f32r, tag="rec")
                    nc.vector.tensor_copy(den_r[:], asbs[h][64:65, :])
                    bcd = pps.tile([64, 512], f32, tag="p1")
                    nc.tensor.matmul(bcd, ones_r[:, 0:64], den_r[:],
                                     start=True, stop=True)
                    bcs = psmall.tile([64, 512], f32, tag="bcs")
                    nc.vector.reciprocal_approx_fast(bcs[:], bcd[:])
                    nc.vector.tensor_mul(oT[h * 64:(h + 1) * 64, ssl],
                                         asbs[h][0:64, :], bcs[:])
                # final chunk: alternate casts between DVE and ScalarE
                # (ACT is idle by now) and out-DMAs between gpsimd and
                # sync, so the tail drains on parallel engines/queues
                for s1 in range(4):
                    s0 = sc * 512 + s1 * 128
                    rs = slice(b * S + s0, b * S + s0 + 128)
                    for n in range(D // 512):
                        nsl = slice(n * 512, (n + 1) * 512)
                        ps = pps.tile([128, 512], f32, tag="p1",
                                      name="oppsumL")
                        nc.tensor.matmul(ps, oT[:, s0:s0 + 128],
                                         wo_sb[:, nsl], start=True, stop=True)
                        ob = pob.tile([128, 512], bf16, tag="ob")
                        if (s1 * 2 + n) % 2:
                            nc.scalar.copy(ob[:], ps)
                            nc.sync.dma_start(out[rs, nsl], ob[:])
                        else:
                            nc.vector.tensor_copy(ob[:], ps)
                            nc.gpsimd.dma_start(out[rs, nsl], ob[:])

            def attention(b, qh, kh, vh_of, last=False, first=False):
                oT = poutT.tile([128, S], bf16, tag="outT", name=f"oT{b}")
                for sc in range(NSC):
                    inline = last and sc == NSC - 1
                    exs = []
                    att = None
                    for t in range(NT):
                        pump()
                        scps = pps.tile([128, HPC, 512], f32, tag="sc")
                        for h in range(HPC):
                            hp = slice(h * 64, (h + 1) * 64)
                            nc.tensor.matmul(scps[:, h, :],
                                             kh[hp, t * 128:(t + 1) * 128],
                                             qh[hp, sc * 512:(sc + 1) * 512],
                                             start=True, stop=True)
                        ex = pexp.tile([128, HPC, 512], bf16, tag="exp")
                        nc.scalar.activation(ex[:], scps[:], Exp, scale=0.125)
                        exs.append(ex)
                        if inline:
                            if att is None:
                                att = [pps.tile([65, 512], f32, tag="att",
                                                name=f"attL{h}")
                                       for h in range(HPC)]
                            vh = vh_of()
                            for h in range(HPC):
                                nc.tensor.matmul(att[h],
                                                 vh[:, t, h * 65:h * 65 + 65],
                                                 ex[:, h, :],
                                                 start=(t == 0),
                                                 stop=(t == NT - 1))
                    if inline:
                        inline_tail(b, sc, att, oT)
                    else:
                        # chunk 0's deferred attn@V races the vh0
                        # projections (lane A); gate it to pop just after
                        # each vh0 t-quad is emitted (item i covers
                        # t = 4*(i//2) .. 4*(i//2)+3)
                        gates = ([20, 21, 25, 26, 29, 30, 34, 35]
                                 if (first and sc == 0) else None)
                        defer_attnv(b, sc, exs, vh_of, oT, av_gates=gates)

            # ---- emission ----
            # (input stripe DMAs were emitted with the consts above, in
            # sync-queue priority order; lands: q_s0 ~10.4, k_s0 ~14.4,
            # then one stripe per ~3.2us.)
            hold = {"vt0": vt0}
            hold["vh0"] = pvh.tile([128, NT, 130], bf16, tag="vh", name="vh0")
            qh0 = pproj.tile([128, S], bf16, tag="projq", name="projq0")
            kh0 = pproj.tile([128, S], bf16, tag="projk", name="projk0")
            proj_stripe(qs0[0], qh0, 0)
            proj_stripe(ks0[0], kh0, 0)

            # lane A thresholds are pump-iterations (~1.05us each from
            # ~16.5us); gate ~= when the stripe's DMA has landed, with
            # FIFO position chosen so each item also pops before its first
            # consumer (sc1 needs qh_s1 by iter 16, kh_s2/s3 by t8/t12 —
            # those are emitted just-in-time and the ~2-tile ACT backlog
            # absorbs the short DMA waits).
            va0 = vh_items(0, hold, [16, 16, 16, 16, 19, 19, 19, 19,
                                     22, 22, 22, 22, 25, 25, 25, 25])
            laneA.extend(proj_stripe_items(lambda: ks0[1], lambda: kh0, 1, 3))
            laneA.extend(proj_stripe_items(lambda: ks0[2], lambda: kh0, 2, 6))
            laneA.extend(proj_stripe_items(lambda: qs0[1], lambda: qh0, 1, 8))
            laneA.extend(proj_stripe_items(lambda: ks0[3], lambda: kh0, 3, 10))
            laneA.extend(va0[0:4])

            def dma_q1k1():
                # k1 right after q1_s0: kh1 is needed at b1's first scores
                # (iter 64) while qh1_s1-3 aren't read until iters 80-112
                qt1 = [None] * NST
                qt1[0] = dma_stripe(qT, 1, "q", 0)
                hold["kt1"] = [dma_stripe(kT, 1, "k", j) for j in range(NST)]
                for j in range(1, NST):
                    qt1[j] = dma_stripe(qT, 1, "q", j)
                hold["qt1"] = qt1
                hold["qh"] = pproj.tile([128, S], bf16, tag="projq",
                                        name="projq1")
                hold["kh"] = pproj.tile([128, S], bf16, tag="projk",
                                        name="projk1")
            laneA.append((17, dma_q1k1))
            laneA.extend(va0[4:12])
            laneA.append((28, lambda: proj_stripe(qs0[2], qh0, 2)))
            laneA.extend(va0[12:16])
            laneA.append((31, lambda: proj_stripe(qs0[3], qh0, 3)))
            laneA.append((34, lambda: proj_stripe(hold["qt1"][0],
                                                  hold["qh"], 0)))
            for j, g in enumerate((37, 40, 43, 47)):
                laneA.append((g, lambda j=j: proj_stripe(hold["kt1"][j],
                                                         hold["kh"], j)))
                if j == 1:
                    def dma_v1():
                        hold["vt1"] = [dma_stripe(vT, 1, "v", jj)
                                       for jj in range(NST)]
                        hold["vh1"] = pvh.tile([128, NT, 130], bf16,
                                               tag="vh", name="vh1")
                    laneA.append((41, dma_v1))
            for j, g in enumerate((50, 53, 56)):
                laneA.append((g, lambda j=j: proj_stripe(hold["qt1"][j + 1],
                                                         hold["qh"], j + 1)))
            laneA.extend(vh_items(1, hold, [59, 59, 59, 59, 62, 62, 62, 62,
                                            65, 65, 65, 65, 68, 68, 68, 68]))

            attention(0, qh0, kh0, lambda: hold["vh0"], first=True)
            attention(1, hold["qh"], hold["kh"], lambda: hold["vh1"])
            wps = pps.tile([128, 512], f32, tag="p1", name="warmtail")

            def warm(n):
                for _ in range(n):
                    nc.tensor.matmul(wps, wq_sb[:, 0, :], wq_sb[:, 0:4, :],
                                     start=True, stop=True,
                                     skip_group_check=True)

            while laneA or laneB:
                if laneA:
                    laneA.pop(0)[1]()
                if laneB:
                    th = laneB.pop(0)[1]()
                    if len(laneB) == 6:   # after last attnV, before norm
                        warm(8)
                    elif len(laneB) == 4:  # after norms, before outproj
                        warm(8)

    nc.compile()
    return nc


def make_in_maps(q, k, v, Wq, bq, Wo):
    bf = ml_dtypes.bfloat16
    xT = {}
    for name, x in (("qT", q), ("kT", k), ("vT", v)):
        xT[name] = np.ascontiguousarray(
            np.asarray(x, np.float32).reshape(BS, D).T).astype(bf)

    in_maps = []
    for c in range(NCORES):
        cols = slice(c * HD, (c + 1) * HD)
        wqc = np.asarray(Wq, np.float32)[:, cols]
        bqc = np.asarray(bq, np.float32)[cols]
        wqve = np.zeros((D, 130), np.float32)
        wqve[:, 0:64] = wqc[:, 0:64]
        wqve[:, 65:129] = wqc[:, 64:128]
        bqve = np.zeros((1, 130), np.float32)
        bqve[0, 0:64] = bqc[0:64]
        bqve[0, 65:129] = bqc[64:128]
        bqve[0, 64] = 1.0
        bqve[0, 129] = 1.0
        in_maps.append({
            "qT": xT["qT"], "kT": xT["kT"], "vT": xT["vT"],
            "wq": np.ascontiguousarray(wqc).astype(bf),
            "wqv": wqve.astype(bf),
            "bq": bqc[None, :].copy(),
            "bqv": bqve,
            "wo": np.ascontiguousarray(np.asarray(Wo, np.float32)[cols, :]).astype(bf),
        })
    return in_maps


def kernel(q, k, v, Wq, bq, Wo, bo):
    import jax
    from concourse.bass_utils import run_bass_kernel_spmd

    try:
        jax.config.update("jax_compilation_cache_dir", "/tmp/jax_bass_cache")
        jax.config.update("jax_persistent_cache_min_entry_size_bytes", -1)
        jax.config.update("jax_persistent_cache_min_compile_time_secs", 0)
    except Exception:
        pass

    if "nc" not in _cache:
        _cache["nc"] = _build()
    nc = _cache["nc"]

    in_maps = make_in_maps(q, k, v, Wq, bq, Wo)
    res = run_bass_kernel_spmd(nc, in_maps, list(range(NCORES)), trace=False)
    acc = np.zeros((BS, D), np.float64)
    for c in range(NCORES):
        acc += res.results[c]["out"].astype(np.float64)
    acc += np.asarray(bo, np.float32)[None, :].astype(np.float64)
    return acc.reshape(B, S, D).astype(np.float32)


# revision 38
# speedup vs baseline: 1.0405x; 1.0405x over previous
"""Multi-head attention (B=2, S=2048, D=1024, H=16, d_k=64) on 8 TRN2 NeuronCores.

Sharding: head-parallel. Core c owns heads (2c, 2c+1) for both batch rows:
 - replicated inputs: qT/kT/vT = x.reshape(B*S, D).T  in bf16, [1024, 4096]
   (D on partitions so the TensorEngine contracts over D with no transposes)
 - per-core weights: Wq columns / Wo rows for its two heads
 - per-core output: partial = attn_out(own heads) @ Wo[own rows]  [4096, 1024] bf16
   The host sums the 8 partials (f32) and adds bo.  No cross-core comm.

Per-core dataflow (bf16 matmuls, f32 PSUM):
 1. qhT/khT [128, 2048] per batch = Wq_c.T @ xT (+bq), computed per
    512-col s-stripe with d-inner accumulation (8 back-to-back matmuls
    hide LDWEIGHTS).  Inputs arrive as [128, 8, 512] stripes — one
    3D-AP DMA per stripe — ordered q_s0, k_s0..s3, v, q_s1..s3 so the
    first scores fire ~14us in instead of ~40us.
    vh [2048, 130] natural = vT.T @ Wqv_c; Wqv has zero-cols / bqv has
    1.0-cols so each head gets a ones column -> attn@V also produces the
    softmax denominators.
 2. scoresT[t,s] = khT.T @ qhT, both heads packed into disjoint PE
    row-groups (K=64).  exp(x/8) on ScalarE from PSUM, bf16 out.
 3. attn@V accumulated over t; row 64 = denominator.  Normalize via
    f32r denominator broadcast (K=1 matmul) + fast reciprocal + mul.
 4. partial[s, :] = outT.T @ Wo_c -> bf16 -> DRAM.

Scheduling: ScalarE (exp, ~142us) and the TensorEngine are both
near-saturated, so emission order keeps ACT fed: each s-chunk's
scores+exp loop is emitted first; its attn@V/normalize/out-proj are
deferred one s-chunk and re-emitted between later score iterations via a
two-lane work queue (lane A: DMA-gated projection work with
earliest-iteration thresholds; lane B: deferred attention, gated only
for chunk 0 where it races the vh0 projections).
"""

import numpy as np
import ml_dtypes

B, S, D, H, DK = 2, 2048, 1024, 16, 64
NCORES = 8
HPC = H // NCORES          # heads per core = 2
BS = B * S                 # 4096
HD = HPC * DK              # 128 = per-core head dims

_cache = {}


def _build():
    import concourse.bass as bass
    import concourse.tile as tile
    from concourse import bacc, mybir

    f32 = mybir.dt.float32
    f32r = mybir.dt.float32r
    bf16 = mybir.dt.bfloat16
    Exp = mybir.ActivationFunctionType.Exp

    nc = bacc.Bacc("TRN2", target_bir_lowering=False, debug=False,
                   num_devices=NCORES)

    qT = nc.declare_dram_parameter("qT", [D, BS], bf16, isOutput=False)
    kT = nc.declare_dram_parameter("kT", [D, BS], bf16, isOutput=False)
    vT = nc.declare_dram_parameter("vT", [D, BS], bf16, isOutput=False)
    wq = nc.declare_dram_parameter("wq", [D, HD], bf16, isOutput=False)
    wqv = nc.declare_dram_parameter("wqv", [D, 130], bf16, isOutput=False)
    bq = nc.declare_dram_parameter("bq", [1, HD], f32, isOutput=False)
    bqv = nc.declare_dram_parameter("bqv", [1, 130], f32, isOutput=False)
    wo = nc.declare_dram_parameter("wo", [HD, D], bf16, isOutput=False)
    out = nc.declare_dram_parameter("out", [BS, D], bf16, isOutput=True)

    ND = D // 128            # 8 d-chunks
    NT = S // 128            # 16 t-chunks per batch
    NSC = S // 512           # 4 s-chunks per batch
    NST = 4                  # 512-col input stripes per batch

    with tile.TileContext(nc) as tc:
        with (
            tc.tile_pool(name="const", bufs=1) as pc,
            tc.tile_pool(name="xin", bufs=12) as pin,
            tc.tile_pool(name="proj", bufs=2) as pproj,
            tc.tile_pool(name="vh", bufs=2) as pvh,
            tc.tile_pool(name="exp", bufs=20) as pexp,
            tc.tile_pool(name="outT", bufs=2) as poutT,
            tc.tile_pool(name="small", bufs=2) as psmall,
            tc.tile_pool(name="ob", bufs=4) as pob,
            tc.tile_pool(name="ps", bufs=2, space="PSUM") as pps,
        ):
            # ---- constants + first input stripes, ALL on the sync queue
            # in priority order (two queues just split HBM bandwidth and
            # delay the critical path): q_s0, bq, wq unblock the first
            # projection; k_s0 the first scores; the rest follows.
            def dma_stripe(src, b, n, j, eng=None):
                """One 512-col stripe of an input, all 8 d-chunks, via a
                single 3D-AP DMA: tile[p, c, s] = src[c*128+p, b*S+j*512+s].
                NB these strided DMAs BLOCK their queue ~3us each (after a
                ~3-deep descriptor ring), so queue order IS the schedule."""
                t = pin.tile([128, ND, 512], bf16, tag="xs",
                             name=f"x{n}{b}{j}")
                ap = src[:, b * S + j * 512:b * S + (j + 1) * 512].rearrange(
                    "(c p) s -> p c s", p=128)
                (eng or nc.sync).dma_start(t[:, :, :], ap)
                return t

            qs0 = [None] * NST
            ks0 = [None] * NST
            qs0[0] = dma_stripe(qT, 0, "q", 0)
            bq_row = pc.tile([1, HD], f32)
            nc.sync.dma_start(bq_row[:], bq[:, :])
            wq_sb = pc.tile([128, ND, HD], bf16)
            nc.sync.dma_start(
                wq_sb[:, :, :],
                wq[:, :].rearrange("(c p) h -> p c h", p=128))
            ks0[0] = dma_stripe(kT, 0, "k", 0)
            bqv_row = pc.tile([1, 130], f32)
            nc.sync.dma_start(bqv_row[:], bqv[:, :])
            wqv_sb = pc.tile([128, ND, 130], bf16)
            nc.sync.dma_start(
                wqv_sb[:, :, :],
                wqv[:, :].rearrange("(c p) h -> p c h", p=128))
            wo_sb = pc.tile([HD, D], bf16)
            nc.sync.dma_start(wo_sb[:], wo[:, :])
            ks0[1] = dma_stripe(kT, 0, "k", 1)
            qs0[1] = dma_stripe(qT, 0, "q", 1)
            ks0[2] = dma_stripe(kT, 0, "k", 2)
            ks0[3] = dma_stripe(kT, 0, "k", 3)
            vt0 = [dma_stripe(vT, 0, "v", j) for j in range(NST)]
            qs0[2] = dma_stripe(qT, 0, "q", 2)
            qs0[3] = dma_stripe(qT, 0, "q", 3)

            ones_f = pc.tile([1, 128], f32)
            nc.vector.memset(ones_f[:], 1.0)
            ones_r = pc.tile([1, 128], f32r)
            nc.vector.tensor_copy(ones_r[:], ones_f[:])
            bq_row_r = pc.tile([1, HD], f32r)
            nc.vector.tensor_copy(bq_row_r[:], bq_row[:])
            bqv_row_r = pc.tile([1, 130], f32r)
            nc.vector.tensor_copy(bqv_row_r[:], bqv_row[:])

            # bq as per-partition column (qhT/khT bias) and broadcast
            # across partitions (vh bias, with the 1.0 ones-columns)
            ps_t = pps.tile([128, 128], f32, tag="p1")
            nc.tensor.matmul(ps_t, bq_row_r[:], ones_r[:],
                             start=True, stop=True)
            bq_col = pc.tile([128, 1], f32)
            nc.vector.tensor_copy(bq_col[:], ps_t[:, 0:1])
            ps_t2 = pps.tile([128, 130], f32, tag="p1")
            nc.tensor.matmul(ps_t2, ones_r[:], bqv_row_r[:],
                             start=True, stop=True)
            bqv_bc = pc.tile([128, 130], f32)
            nc.vector.tensor_copy(bqv_bc[:], ps_t2[:])

            def proj_stripe(xs, sb, j):
                """qh/kh for one 512-col stripe: 8 d-inner accumulating
                matmuls (LDWEIGHTS fully hidden) + bias."""
                ps = pps.tile([128, 512], f32, tag="p1", name="pjs")
                for d in range(ND):
                    nc.tensor.matmul(ps, wq_sb[:, d, :], xs[:, d, :],
                                     start=(d == 0), stop=(d == ND - 1))
                nc.vector.tensor_scalar_add(sb[:, j * 512:(j + 1) * 512],
                                            ps, bq_col[:])

            def proj_stripe_items(xs_of, sb_of, j, gate):
                """proj_stripe split into two 4-MM lane-A items so a
                single pump never displaces scores by more than ~1us
                (ACT's PSUM backlog is only 2 tiles deep)."""
                cell = {}

                def p_a():
                    ps = pps.tile([128, 512], f32, tag="p1", name="pjs")
                    xs = xs_of()
                    for d in range(4):
                        nc.tensor.matmul(ps, wq_sb[:, d, :], xs[:, d, :],
                                         start=(d == 0), stop=False)
                    cell["ps"] = ps

                def p_b():
                    ps = cell["ps"]
                    xs = xs_of()
                    for d in range(4, ND):
                        nc.tensor.matmul(ps, wq_sb[:, d, :], xs[:, d, :],
                                         start=False, stop=(d == ND - 1))
                    nc.vector.tensor_scalar_add(
                        sb_of()[:, j * 512:(j + 1) * 512], ps, bq_col[:])
                return [(gate, p_a), (gate, p_b)]

            def vh_items(b, hold, gates):
                items = []
                for t in range(NT):
                    def tt(t=t):
                        ps = pps.tile([128, 130], f32,
                                      tag="p1", name=f"pvh{t}")
                        vt = hold["vt" + str(b)][t // 4]
                        c0 = (t % 4) * 128
                        for d in range(ND):
                            nc.tensor.matmul(ps, vt[:, d, c0:c0 + 128],
                                             wqv_sb[:, d, :],
                                             start=(d == 0), stop=(d == ND - 1))
                        nc.vector.tensor_add(hold["vh" + str(b)][:, t, :],
                                             ps[:], bqv_bc[:])
                    items.append((gates[t], tt))
                return items

            # ---- two-lane deferred work queue ----
            laneA = []   # (min_iter, thunk): DMA-gated projection work
            laneB = []   # (min_iter, thunk): deferred attention work
            it = [0]

            def pump():
                popped = 0
                if laneA and laneA[0][0] <= it[0]:
                    laneA.pop(0)[1]()
                    popped = 1
                for _ in range(2 - popped):
                    if laneB and laneB[0][0] <= it[0]:
                        laneB.pop(0)[1]()
                it[0] += 1

            def defer_attnv(b, sc, exs, vh_of, oT, av_gates=None):
                ssl = slice(sc * 512, (sc + 1) * 512)
                cell = {}
                # item i = one head (i%2), two consecutive t-tiles: both
                # MMs accumulate into the SAME PSUM bank back-to-back,
                # which is the only pattern where LDWEIGHTS stays hidden
                for i in range(NT):
                    def av(i=i):
                        h = i % 2
                        if i == 0:
                            cell["att"] = [
                                pps.tile([65, 512], f32, tag="att",
                                         name=f"att{b}{sc}{hh}")
                                for hh in range(HPC)]
                        vh = vh_of()
                        for t in (2 * (i // 2), 2 * (i // 2) + 1):
                            nc.tensor.matmul(cell["att"][h],
                                             vh[:, t, h * 65:h * 65 + 65],
                                             exs[t][:, h, :],
                                             start=(t == 0), stop=(t == NT - 1))
                    laneB.append((av_gates[i] if av_gates else 0, av))

                # norm and out-proj split into ~1us items so a single pump
                # never displaces scores past the 2-tile ACT backlog
                def norm_h(h):
                    asb = psmall.tile([65, 512], f32, tag="asb",
                                      bufs=4, name=f"asb{h}")
                    nc.vector.tensor_copy(asb[:], cell["att"][h][:])
                    den_r = psmall.tile([1, 512], f32r, tag="rec")
                    nc.vector.tensor_copy(den_r[:], asb[64:65, :])
                    bcd = pps.tile([64, 512], f32, tag="p1")
                    nc.tensor.matmul(bcd, ones_r[:, 0:64], den_r[:],
                                     start=True, stop=True)
                    bcs = psmall.tile([64, 512], f32, tag="bcs")
                    nc.vector.reciprocal_approx_fast(bcs[:], bcd[:])
                    nc.vector.tensor_mul(oT[h * 64:(h + 1) * 64, ssl],
                                         asb[0:64, :], bcs[:])
                laneB.append((0, lambda: norm_h(0)))
                laneB.append((0, lambda: norm_h(1)))

                for g in range(4):
                    def op(g=g):
                        for u in range(2):
                            s1 = sc * 4 + g
                            n = u
                            s0 = s1 * 128
                            rs = slice(b * S + s0, b * S + s0 + 128)
                            nsl = slice(n * 512, (n + 1) * 512)
                            ps = pps.tile([128, 512], f32,
                                          tag="p1", name="oppsum")
                            nc.tensor.matmul(ps, oT[:, s0:s0 + 128],
                                             wo_sb[:, nsl],
                                             start=True, stop=True)
                            ob = pob.tile([128, 512], bf16, tag="ob")
                            nc.vector.tensor_copy(ob[:], ps)
                            nc.gpsimd.dma_start(out[rs, nsl], ob[:])
                    laneB.append((0, op))

            def inline_tail(b, sc, att, oT):
                ssl = slice(sc * 512, (sc + 1) * 512)
                asbs = []
                for h in range(HPC):
                    asb = psmall.tile([65, 512], f32, tag="asb",
                                      bufs=4, name=f"asbL{h}")
                    nc.vector.tensor_copy(asb[:], att[h][:])
                    asbs.append(asb)
                for h in range(HPC):
                    den_r = psmall.tile([1, 512], f32r, tag="rec")
                    nc.vector.tensor_copy(den_r[:], asbs[h][64:65, :])
                    bcd = pps.tile([64, 512], f32, tag="p1")
                    nc.tensor.matmul(bcd, ones_r[:, 0:64], den_r[:],
                                     start=True, stop=True)
                    bcs = psmall.tile([64, 512], f32, tag="bcs")
                    nc.vector.reciprocal_approx_fast(bcs[:], bcd[:])
                    nc.vector.tensor_mul(oT[h * 64:(h + 1) * 64, ssl],
                                         asbs[h][0:64, :], bcs[:])
                # final chunk: alternate out-DMAs between gpsimd and sync
                # so the tail writes drain on two queues in parallel
                for s1 in range(4):
                    s0 = sc * 512 + s1 * 128
                    rs = slice(b * S + s0, b * S + s0 + 128)
                    for n in range(D // 512):
                        nsl = slice(n * 512, (n + 1) * 512)
                        ps = pps.tile([128, 512], f32, tag="p1",
                                      name="oppsumL")
                        nc.tensor.matmul(ps, oT[:, s0:s0 + 128],
                                         wo_sb[:, nsl], start=True, stop=True)
                        ob = pob.tile([128, 512], bf16, tag="ob")
                        nc.vector.tensor_copy(ob[:], ps)
                        eng = nc.sync if (s1 * 2 + n) % 2 else nc.gpsimd
                        eng.dma_start(out[rs, nsl], ob[:])

            def attention(b, qh, kh, vh_of, last=False, first=False):
                oT = poutT.tile([128, S], bf16, tag="outT", name=f"oT{b}")
                for sc in range(NSC):
                    inline = last and sc == NSC - 1
                    exs = []
                    att = None
                    for t in range(NT):
                        pump()
                        scps = pps.tile([128, HPC, 512], f32, tag="sc")
                        for h in range(HPC):
                            hp = slice(h * 64, (h + 1) * 64)
                            nc.tensor.matmul(scps[:, h, :],
                                             kh[hp, t * 128:(t + 1) * 128],
                                             qh[hp, sc * 512:(sc + 1) * 512],
                                             start=True, stop=True)
                        ex = pexp.tile([128, HPC, 512], bf16, tag="exp")
                        nc.scalar.activation(ex[:], scps[:], Exp, scale=0.125)
                        exs.append(ex)
                        if inline:
                            if att is None:
                                att = [pps.tile([65, 512], f32, tag="att",
                                                name=f"attL{h}")
                                       for h in range(HPC)]
                            vh = vh_of()
                            for h in range(HPC):
                                nc.tensor.matmul(att[h],
                                                 vh[:, t, h * 65:h * 65 + 65],
                                                 ex[:, h, :],
                                                 start=(t == 0),
                                                 stop=(t == NT - 1))
                    if inline:
                        inline_tail(b, sc, att, oT)
                    else:
                        # chunk 0's deferred attn@V races the vh0
                        # projections (lane A); gate it to pop just after
                        # each vh0 t-pair is emitted (item i covers
                        # t = 2*(i//2), 2*(i//2)+1)
                        gates = ([18, 18, 20, 20, 23, 23, 25, 25,
                                  27, 27, 29, 29, 32, 32, 34, 34]
                                 if (first and sc == 0) else None)
                        defer_attnv(b, sc, exs, vh_of, oT, av_gates=gates)

            # ---- emission ----
            # (input stripe DMAs were emitted with the consts above, in
            # sync-queue priority order; lands: q_s0 ~10.4, k_s0 ~14.4,
            # then one stripe per ~3.2us.)
            hold = {"vt0": vt0}
            hold["vh0"] = pvh.tile([128, NT, 130], bf16, tag="vh", name="vh0")
            qh0 = pproj.tile([128, S], bf16, tag="projq", name="projq0")
            kh0 = pproj.tile([128, S], bf16, tag="projk", name="projk0")
            proj_stripe(qs0[0], qh0, 0)
            proj_stripe(ks0[0], kh0, 0)

            # lane A thresholds are pump-iterations (~1.05us each from
            # ~16.5us); gate ~= when the stripe's DMA has landed, with
            # FIFO position chosen so each item also pops before its first
            # consumer (sc1 needs qh_s1 by iter 16, kh_s2/s3 by t8/t12 —
            # those are emitted just-in-time and the ~2-tile ACT backlog
            # absorbs the short DMA waits).
            va0 = vh_items(0, hold, [16, 16, 16, 16, 19, 19, 19, 19,
                                     22, 22, 22, 22, 25, 25, 25, 25])
            laneA.extend(proj_stripe_items(lambda: ks0[1], lambda: kh0, 1, 3))
            laneA.extend(proj_stripe_items(lambda: ks0[2], lambda: kh0, 2, 6))
            laneA.extend(proj_stripe_items(lambda: qs0[1], lambda: qh0, 1, 8))
            laneA.extend(proj_stripe_items(lambda: ks0[3], lambda: kh0, 3, 10))
            laneA.extend(va0[0:4])

            def dma_q1k1():
                hold["qt1"] = [dma_stripe(qT, 1, "q", j) for j in range(NST)]
                hold["kt1"] = [dma_stripe(kT, 1, "k", j) for j in range(NST)]
                hold["qh"] = pproj.tile([128, S], bf16, tag="projq",
                                        name="projq1")
                hold["kh"] = pproj.tile([128, S], bf16, tag="projk",
                                        name="projk1")
            laneA.append((17, dma_q1k1))
            laneA.extend(va0[4:12])
            laneA.append((28, lambda: proj_stripe(qs0[2], qh0, 2)))
            laneA.extend(va0[12:16])
            laneA.append((31, lambda: proj_stripe(qs0[3], qh0, 3)))
            for j, g in enumerate((34, 37, 40, 43)):
                laneA.append((g, lambda j=j: proj_stripe(hold["qt1"][j],
                                                         hold["qh"], j)))
            for j, g in enumerate((47, 50, 53, 56)):
                laneA.append((g, lambda j=j: proj_stripe(hold["kt1"][j],
                                                         hold["kh"], j)))
                if j == 1:
                    def dma_v1():
                        hold["vt1"] = [dma_stripe(vT, 1, "v", jj)
                                       for jj in range(NST)]
                        hold["vh1"] = pvh.tile([128, NT, 130], bf16,
                                               tag="vh", name="vh1")
                    laneA.append((51, dma_v1))
            laneA.extend(vh_items(1, hold, [59, 59, 59, 59, 62, 62, 62, 62,
                                            65, 65, 65, 65, 68, 68, 68, 68]))

            attention(0, qh0, kh0, lambda: hold["vh0"], first=True)
            attention(1, hold["qh"], hold["kh"], lambda: hold["vh1"])
            wps = pps.tile([128, 512], f32, tag="p1", name="warmtail")

            def warm(n):
                for _ in range(n):
                    nc.tensor.matmul(wps, wq_sb[:, 0, :], wq_sb[:, 0:4, :],
                                     start=True, stop=True,
                                     skip_group_check=True)

            while laneA or laneB:
                if laneA:
                    laneA.pop(0)[1]()
                if laneB:
                    th = laneB.pop(0)[1]()
                    if len(laneB) == 6:   # after last attnV, before norm
                        warm(8)
                    elif len(laneB) == 4:  # after norms, before outproj
                        warm(8)

    nc.compile()
    return nc


def make_in_maps(q, k, v, Wq, bq, Wo):
    bf = ml_dtypes.bfloat16
    xT = {}
    for name, x in (("qT", q), ("kT", k), ("vT", v)):
        xT[name] = np.ascontiguousarray(
            np.asarray(x, np.float32).reshape(BS, D).T).astype(bf)

    in_maps = []
    for c in range(NCORES):
        cols = slice(c * HD, (c + 1) * HD)
        wqc = np.asarray(Wq, np.float32)[:, cols]
        bqc = np.asarray(bq, np.float32)[cols]
        wqve = np.zeros((D, 130), np.float32)
        wqve[:, 0:64] = wqc[:, 0:64]
        wqve[:, 65:129] = wqc[:, 64:128]
        bqve = np.zeros((1, 130), np.float32)
        bqve[0, 0:64] = bqc[0:64]
        bqve[0, 65:129] = bqc[64:128]
        bqve[0, 64] = 1.0
        bqve[0, 129] = 1.0
        in_maps.append({
            "qT": xT["qT"], "kT": xT["kT"], "vT": xT["vT"],
            "wq": np.ascontiguousarray(wqc).astype(bf),
            "wqv": wqve.astype(bf),
            "bq": bqc[None, :].copy(),
            "bqv": bqve,
            "wo": np.ascontiguousarray(np.asarray(Wo, np.float32)[cols, :]).astype(bf),
        })
    return in_maps


def kernel(q, k, v, Wq, bq, Wo, bo):
    import jax
    from concourse.bass_utils import run_bass_kernel_spmd

    try:
        jax.config.update("jax_compilation_cache_dir", "/tmp/jax_bass_cache")
        jax.config.update("jax_persistent_cache_min_entry_size_bytes", -1)
        jax.config.update("jax_persistent_cache_min_compile_time_secs", 0)
    except Exception:
        pass

    if "nc" not in _cache:
        _cache["nc"] = _build()
    nc = _cache["nc"]

    in_maps = make_in_maps(q, k, v, Wq, bq, Wo)
    res = run_bass_kernel_spmd(nc, in_maps, list(range(NCORES)), trace=False)
    acc = np.zeros((BS, D), np.float64)
    for c in range(NCORES):
        acc += res.results[c]["out"].astype(np.float64)
    acc += np.asarray(bo, np.float32)[None, :].astype(np.float64)
    return acc.reshape(B, S, D).astype(np.float32)
